# revision 8
# baseline (speedup 1.0000x reference)
"""Top-1 nearest-neighbor retrieval kernel for Trainium2 (8 NeuronCores).

Reference computation:
    dis = sum((db_vel - in_vel)**2, axis=1)   # [N]
    ind = argmin(dis)
    out = pred_vel[ind][None, :]

Strategy (memory-bound):
  - Shard db_vel row-wise: 100000 rows -> 8 cores x 12500 rows.
  - Each core streams its 12500x1056 f32 shard from HBM; chunk sizes ramp
    1-2-4 tiles at the start and 4-2-1 at the end (7-tile / 3.78 MB chunks
    in steady state) so compute starts early and drains fast.
  - Distances: each 128-row tile takes ONE fused custom DVE op
        out = (db - q_bcast)**2 ; accum_out = row-sum(out)
    (registered at runtime as SQDIFF_ACC_ANT) - a single 1x-rate vector
    pass (~123 us/core) under the ~130 us DMA stream, so the kernel is
    DMA-bound at ~400 GB/s/core.
  - Each core writes a tiny [128, 98] distance tile; the host does the
    final argmin over 100k scalars and gathers the pred_vel row (pred_vel
    never touches the device - only one of its rows is ever read).
"""

import numpy as np

N_DB = 100000
D_IN = 1056
N_CORES = 8
ROWS = N_DB // N_CORES          # 12500 rows per core
P = 128                         # SBUF partitions
NTILES = (ROWS + P - 1) // P    # 98 tile-columns (97 full + one 84-row)
# Chunk schedule (tiles per DMA): fast ramp-in and drain-out.
CHUNKS = [1, 2, 4] + [7] * 12 + [4, 2, 1]
assert sum(CHUNKS) == NTILES
MAXCHUNK = max(CHUNKS)

_CACHE = {}


def _get_sqdiff_op():
    """Register (once) a fused custom DVE op:
        out = (in0 - in1)**2 ; accum_out = sum(out, axis=free)
    One 1x-rate vector pass computes the whole squared distance."""
    if "op" in _CACHE:
        return _CACHE["op"]

    from operator import add

    from concourse import dve_ops
    from concourse.dve_spec import Spec, Src0, Src1, Zero, _has_src1, lower, sq
    from concourse.dve_uop import DveOpSpec

    NAME = "SQDIFF_ACC_ANT"

    def _ref(in0, in1, c0, c1, c2):
        b = ((in0.astype(np.float32) - in1) ** 2).astype(np.float32)
        return b, b.reshape(b.shape[0], -1).sum(axis=-1, keepdims=True)

    spec = Spec(body=sq(Src0 - Src1), accum=add, accum_init=Zero, reference=_ref)

    existing = {o.name: o for o in dve_ops.OPS}
    if NAME in existing:
        op = existing[NAME]
    else:
        row = max(dve_ops._SUB_OPCODE_FOR_NAME.values()) + 1
        dve_ops._SUB_OPCODE_FOR_NAME[NAME] = row
        shas = {}
        for ver in ("v3", "v4"):
            try:
                uops = lower(spec, ver=ver)
                shas[ver] = DveOpSpec(
                    name=NAME, opcode=row, uops=uops, rd1_en=_has_src1(spec)
                ).sha(ver)
            except Exception:
                pass
        op = dve_ops.DveOp(NAME, spec, subdim=False, uops_sha=shas)
        dve_ops.OPS.append(op)
        dve_ops.CUSTOM_DVE_SPECS[NAME] = spec

    _CACHE["op"] = op
    return op


def _build():
    """Trace + compile the per-core Bass program (cached)."""
    if "nc" in _CACHE:
        return _CACHE["nc"]

    import concourse.bacc as bacc
    import concourse.mybir as mybir
    from concourse.tile import TileContext

    sqdiff = _get_sqdiff_op()
    f32 = mybir.dt.float32

    nc = bacc.Bacc(trn_type="TRN2", debug=False)
    db = nc.dram_tensor("db", [ROWS, D_IN], f32, kind="ExternalInput").ap()
    q = nc.dram_tensor("q", [P, D_IN], f32, kind="ExternalInput").ap()
    dis = nc.dram_tensor("dis", [P, NTILES], f32, kind="ExternalOutput").ap()

    with TileContext(nc) as tc:
        with (
            tc.tile_pool(name="const", bufs=1) as cpool,
            tc.tile_pool(name="chunks", bufs=4) as chpool,
            tc.tile_pool(name="work", bufs=3) as wpool,
            tc.tile_pool(name="acc", bufs=1) as apool,
        ):
            # Query arrives pre-broadcast to all 128 partitions (host-side).
            qb = cpool.tile([P, D_IN], f32)
            nc.scalar.dma_start(out=qb[:, :], in_=q[:, :])

            dis_sb = apool.tile([P, NTILES], f32)

            t0 = 0
            dma_engines = [nc.sync, nc.scalar]
            for ci, sz in enumerate(CHUNKS):
                r0 = t0 * P
                last = t0 + sz == NTILES
                eng = dma_engines[ci % 2]
                chunk = chpool.tile([P, MAXCHUNK, D_IN], f32, tag="chunk")
                if not last:
                    # Sub-tile a, partition p holds row r0 + a*128 + p.
                    eng.dma_start(
                        out=chunk[:, :sz, :],
                        in_=db[r0 : r0 + sz * P, :].rearrange("(a p) d -> p a d", p=P),
                    )
                    nrows = [P] * sz
                else:
                    # Final tile(s): ends with the 84-row remainder.
                    full = (ROWS - r0) // P
                    tail = ROWS - r0 - full * P
                    if full:
                        eng.dma_start(
                            out=chunk[:, :full, :],
                            in_=db[r0 : r0 + full * P, :].rearrange(
                                "(a p) d -> p a d", p=P
                            ),
                        )
                    eng.dma_start(
                        out=chunk[:tail, full, :],
                        in_=db[r0 + full * P : ROWS, :],
                    )
                    nrows = [P] * full + [tail]

                for a in range(sz):
                    t = t0 + a
                    rows = nrows[a]
                    sq_v = wpool.tile([P, D_IN], f32, tag="sqv")
                    nc.vector._custom_dve(
                        sqdiff,
                        out=sq_v[:rows, :],
                        in0=chunk[:rows, a, :],
                        in1=qb[:rows, :],
                        accum_out=dis_sb[:rows, t : t + 1],
                    )
                t0 += sz

            # Ship finished distance columns early; only the last few columns
            # remain for the drain. (Valid entries cover every db row; the
            # few never-written slots are ignored by the host.)
            nc.scalar.dma_start(out=dis[:, :64], in_=dis_sb[:, :64])
            nc.scalar.dma_start(out=dis[:, 64:91], in_=dis_sb[:, 64:91])
            nc.scalar.dma_start(out=dis[:, 91:], in_=dis_sb[:, 91:])

    nc.compile()
    _CACHE["nc"] = nc
    return nc


def _run(in_maps, **kwargs):
    from concourse.bass_utils import run_bass_kernel_spmd

    nc = _build()
    return run_bass_kernel_spmd(nc, in_maps, core_ids=list(range(N_CORES)), **kwargs)


def make_in_maps(in_vel, db_vel):
    qb = np.ascontiguousarray(
        np.broadcast_to(np.asarray(in_vel, dtype=np.float32), (P, D_IN))
    )
    return [
        {
            "db": np.ascontiguousarray(db_vel[c * ROWS : (c + 1) * ROWS]),
            "q": qb,
        }
        for c in range(N_CORES)
    ]


def dense_distances(results):
    """[N_CORES, ROWS] distances in db row order (mirrors the reference).
    dis[p, t] is the distance of shard row t*128 + p."""
    out = np.empty((N_CORES, ROWS), dtype=np.float32)
    for c in range(N_CORES):
        out[c] = np.transpose(results[c]["dis"]).reshape(-1)[:ROWS]
    return out


def postprocess(results, pred_vel):
    flat = dense_distances(results).reshape(-1)
    ind = int(np.argmin(flat))
    return pred_vel[ind][None, :]


def kernel(in_vel, db_vel, pred_vel):
    res = _run(make_in_maps(in_vel, db_vel))
    return postprocess(res.results, pred_vel)


# revision 9
# speedup vs baseline: 1.0375x; 1.0375x over previous
"""Top-1 nearest-neighbor retrieval kernel for Trainium2 (8 NeuronCores).

Reference computation:
    dis = sum((db_vel - in_vel)**2, axis=1)   # [N]
    ind = argmin(dis)
    out = pred_vel[ind][None, :]

Strategy (memory-bound):
  - Shard db_vel row-wise: 100000 rows -> 8 cores x 12500 rows.
  - Each core streams its 12500x1056 f32 shard from HBM; chunk sizes ramp
    1-2-4 tiles at the start and 4-2-1 at the end (7-tile / 3.78 MB chunks
    in steady state) so compute starts early and drains fast.
  - Distances: each 128-row tile takes ONE fused custom DVE op
        out = (db - q_bcast)**2 ; accum_out = row-sum(out)
    (registered at runtime as SQDIFF_ACC_ANT) - a single 1x-rate vector
    pass (~123 us/core) under the ~130 us DMA stream, so the kernel is
    DMA-bound at ~400 GB/s/core.
  - Each core writes a tiny [128, 98] distance tile; the host does the
    final argmin over 100k scalars and gathers the pred_vel row (pred_vel
    never touches the device - only one of its rows is ever read).
"""

import numpy as np

N_DB = 100000
D_IN = 1056
N_CORES = 8
ROWS = N_DB // N_CORES          # 12500 rows per core
P = 128                         # SBUF partitions
NTILES = (ROWS + P - 1) // P    # 98 tile-columns (97 full + one 84-row)
# Chunk schedule (tiles per DMA): fast ramp-in and drain-out.
CHUNKS = [1, 2, 4] + [7] * 12 + [4, 2, 1]
assert sum(CHUNKS) == NTILES
MAXCHUNK = max(CHUNKS)

_CACHE = {}


def _get_sqdiff_op():
    """Register (once) a fused custom DVE op:
        out = (in0 - in1)**2 ; accum_out = sum(out, axis=free)
    One 1x-rate vector pass computes the whole squared distance."""
    if "op" in _CACHE:
        return _CACHE["op"]

    from operator import add

    from concourse import dve_ops
    from concourse.dve_spec import Spec, Src0, Src1, Zero, _has_src1, lower, sq
    from concourse.dve_uop import DveOpSpec

    NAME = "SQDIFF_ACC_ANT"

    def _ref(in0, in1, c0, c1, c2):
        b = ((in0.astype(np.float32) - in1) ** 2).astype(np.float32)
        return b, b.reshape(b.shape[0], -1).sum(axis=-1, keepdims=True)

    spec = Spec(body=sq(Src0 - Src1), accum=add, accum_init=Zero, reference=_ref)

    existing = {o.name: o for o in dve_ops.OPS}
    if NAME in existing:
        op = existing[NAME]
    else:
        row = max(dve_ops._SUB_OPCODE_FOR_NAME.values()) + 1
        dve_ops._SUB_OPCODE_FOR_NAME[NAME] = row
        shas = {}
        for ver in ("v3", "v4"):
            try:
                uops = lower(spec, ver=ver)
                shas[ver] = DveOpSpec(
                    name=NAME, opcode=row, uops=uops, rd1_en=_has_src1(spec)
                ).sha(ver)
            except Exception:
                pass
        op = dve_ops.DveOp(NAME, spec, subdim=False, uops_sha=shas)
        dve_ops.OPS.append(op)
        dve_ops.CUSTOM_DVE_SPECS[NAME] = spec

    _CACHE["op"] = op
    return op


def _build():
    """Trace + compile the per-core Bass program (cached)."""
    if "nc" in _CACHE:
        return _CACHE["nc"]

    import concourse.bacc as bacc
    import concourse.mybir as mybir
    from concourse.tile import TileContext

    sqdiff = _get_sqdiff_op()
    f32 = mybir.dt.float32

    nc = bacc.Bacc(trn_type="TRN2", debug=False)
    db = nc.dram_tensor("db", [ROWS, D_IN], f32, kind="ExternalInput").ap()
    q = nc.dram_tensor("q", [P, D_IN], f32, kind="ExternalInput").ap()
    dis = nc.dram_tensor("dis", [P, NTILES], f32, kind="ExternalOutput").ap()

    with TileContext(nc) as tc:
        with (
            tc.tile_pool(name="const", bufs=1) as cpool,
            tc.tile_pool(name="chunks", bufs=4) as chpool,
            tc.tile_pool(name="work", bufs=3) as wpool,
            tc.tile_pool(name="acc", bufs=1) as apool,
        ):
            # Query arrives pre-broadcast to all 128 partitions (host-side).
            qb = cpool.tile([P, D_IN], f32)
            nc.scalar.dma_start(out=qb[:, :], in_=q[:, :])

            dis_sb = apool.tile([P, NTILES], f32)

            t0 = 0
            for sz in CHUNKS:
                r0 = t0 * P
                last = t0 + sz == NTILES
                chunk = chpool.tile([P, MAXCHUNK, D_IN], f32, tag="chunk")
                if not last:
                    # Sub-tile a, partition p holds row r0 + a*128 + p.
                    nc.sync.dma_start(
                        out=chunk[:, :sz, :],
                        in_=db[r0 : r0 + sz * P, :].rearrange("(a p) d -> p a d", p=P),
                    )
                    nrows = [P] * sz
                else:
                    # Final tile(s): ends with the 84-row remainder.
                    full = (ROWS - r0) // P
                    tail = ROWS - r0 - full * P
                    if full:
                        nc.sync.dma_start(
                            out=chunk[:, :full, :],
                            in_=db[r0 : r0 + full * P, :].rearrange(
                                "(a p) d -> p a d", p=P
                            ),
                        )
                    nc.sync.dma_start(
                        out=chunk[:tail, full, :],
                        in_=db[r0 + full * P : ROWS, :],
                    )
                    nrows = [P] * full + [tail]

                for a in range(sz):
                    t = t0 + a
                    rows = nrows[a]
                    sq_v = wpool.tile([P, D_IN], f32, tag="sqv")
                    nc.vector._custom_dve(
                        sqdiff,
                        out=sq_v[:rows, :],
                        in0=chunk[:rows, a, :],
                        in1=qb[:rows, :],
                        accum_out=dis_sb[:rows, t : t + 1],
                    )
                t0 += sz

            # Ship finished distance columns early; only the last few columns
            # remain for the drain. (Valid entries cover every db row; the
            # few never-written slots are ignored by the host.)
            nc.scalar.dma_start(out=dis[:, :64], in_=dis_sb[:, :64])
            nc.scalar.dma_start(out=dis[:, 64:91], in_=dis_sb[:, 64:91])
            nc.scalar.dma_start(out=dis[:, 91:], in_=dis_sb[:, 91:])

    nc.compile()
    _CACHE["nc"] = nc
    return nc


def _run(in_maps, **kwargs):
    from concourse.bass_utils import run_bass_kernel_spmd

    nc = _build()
    return run_bass_kernel_spmd(nc, in_maps, core_ids=list(range(N_CORES)), **kwargs)


def make_in_maps(in_vel, db_vel):
    qb = np.ascontiguousarray(
        np.broadcast_to(np.asarray(in_vel, dtype=np.float32), (P, D_IN))
    )
    return [
        {
            "db": np.ascontiguousarray(db_vel[c * ROWS : (c + 1) * ROWS]),
            "q": qb,
        }
        for c in range(N_CORES)
    ]


def dense_distances(results):
    """[N_CORES, ROWS] distances in db row order (mirrors the reference).
    dis[p, t] is the distance of shard row t*128 + p."""
    out = np.empty((N_CORES, ROWS), dtype=np.float32)
    for c in range(N_CORES):
        out[c] = np.transpose(results[c]["dis"]).reshape(-1)[:ROWS]
    return out


def postprocess(results, pred_vel):
    flat = dense_distances(results).reshape(-1)
    ind = int(np.argmin(flat))
    return pred_vel[ind][None, :]


def kernel(in_vel, db_vel, pred_vel):
    res = _run(make_in_maps(in_vel, db_vel))
    return postprocess(res.results, pred_vel)


# revision 13
# speedup vs baseline: 1.0533x; 1.0153x over previous
"""Top-1 nearest-neighbor retrieval kernel for Trainium2 (8 NeuronCores).

Reference computation:
    dis = sum((db_vel - in_vel)**2, axis=1)   # [N]
    ind = argmin(dis)
    out = pred_vel[ind][None, :]

Strategy (memory-bound):
  - Shard db_vel row-wise: 100000 rows -> 8 cores x 12500 rows.
  - Each core streams its 12500x1056 f32 shard from HBM; chunk sizes ramp
    1-2-4 tiles at the start and 4-2-1 at the end (7-tile / 3.78 MB chunks
    in steady state) so compute starts early and drains fast.
  - Distances: each 128-row tile takes ONE fused custom DVE op
        out = (db - q_bcast)**2 ; accum_out = row-sum(out)
    (registered at runtime as SQDIFF_ACC_ANT) - a single 1x-rate vector
    pass (~123 us/core) under the ~130 us DMA stream, so the kernel is
    DMA-bound at ~400 GB/s/core.
  - Each core writes a tiny [128, 98] distance tile; the host does the
    final argmin over 100k scalars and gathers the pred_vel row (pred_vel
    never touches the device - only one of its rows is ever read).
"""

import numpy as np

N_DB = 100000
D_IN = 1056
N_CORES = 8
ROWS = N_DB // N_CORES          # 12500 rows per core
P = 128                         # SBUF partitions
NTILES = (ROWS + P - 1) // P    # 98 tile-columns (97 full + one 84-row)
# Chunk schedule (tiles per DMA): fast ramp-in and drain-out.
CHUNKS = [1, 2, 4] + [7] * 12 + [4, 2, 1]
assert sum(CHUNKS) == NTILES
MAXCHUNK = max(CHUNKS)

_CACHE = {}


def _get_sqdiff_op():
    """Register (once) a fused custom DVE op:
        out = (in0 - in1)**2 ; accum_out = sum(out, axis=free)
    One 1x-rate vector pass computes the whole squared distance."""
    if "op" in _CACHE:
        return _CACHE["op"]

    from operator import add

    from concourse import dve_ops
    from concourse.dve_spec import Spec, Src0, Src1, Zero, _has_src1, lower, sq
    from concourse.dve_uop import DveOpSpec

    NAME = "SQDIFF_ACC_ANT"

    def _ref(in0, in1, c0, c1, c2):
        b = ((in0.astype(np.float32) - in1) ** 2).astype(np.float32)
        return b, b.reshape(b.shape[0], -1).sum(axis=-1, keepdims=True)

    spec = Spec(body=sq(Src0 - Src1), accum=add, accum_init=Zero, reference=_ref)

    existing = {o.name: o for o in dve_ops.OPS}
    if NAME in existing:
        op = existing[NAME]
    else:
        row = max(dve_ops._SUB_OPCODE_FOR_NAME.values()) + 1
        dve_ops._SUB_OPCODE_FOR_NAME[NAME] = row
        shas = {}
        for ver in ("v3", "v4"):
            try:
                uops = lower(spec, ver=ver)
                shas[ver] = DveOpSpec(
                    name=NAME, opcode=row, uops=uops, rd1_en=_has_src1(spec)
                ).sha(ver)
            except Exception:
                pass
        op = dve_ops.DveOp(NAME, spec, subdim=False, uops_sha=shas)
        dve_ops.OPS.append(op)
        dve_ops.CUSTOM_DVE_SPECS[NAME] = spec

    _CACHE["op"] = op
    return op


def _build():
    """Trace + compile the per-core Bass program (cached)."""
    if "nc" in _CACHE:
        return _CACHE["nc"]

    import concourse.bacc as bacc
    import concourse.mybir as mybir
    from concourse.tile import TileContext

    sqdiff = _get_sqdiff_op()
    f32 = mybir.dt.float32

    nc = bacc.Bacc(trn_type="TRN2", debug=False)
    db = nc.dram_tensor("db", [ROWS, D_IN], f32, kind="ExternalInput").ap()
    q = nc.dram_tensor("q", [P, D_IN], f32, kind="ExternalInput").ap()
    dis = nc.dram_tensor("dis", [P, NTILES], f32, kind="ExternalOutput").ap()

    with TileContext(nc) as tc:
        with (
            tc.tile_pool(name="const", bufs=1) as cpool,
            tc.tile_pool(name="chunks", bufs=4) as chpool,
            tc.tile_pool(name="work", bufs=3) as wpool,
            tc.tile_pool(name="acc", bufs=1) as apool,
        ):
            # Query arrives pre-broadcast to all 128 partitions (host-side).
            qb = cpool.tile([P, D_IN], f32)
            nc.scalar.dma_start(out=qb[:, :], in_=q[:, :])

            dis_sb = apool.tile([P, NTILES], f32)

            t0 = 0
            for sz in CHUNKS:
                r0 = t0 * P
                last = t0 + sz == NTILES
                chunk = chpool.tile([P, MAXCHUNK, D_IN], f32, tag="chunk")
                if not last:
                    # Partition p holds rows r0 + p*sz .. r0 + p*sz + sz-1.
                    nc.sync.dma_start(
                        out=chunk[:, :sz, :],
                        in_=db[r0 : r0 + sz * P, :].rearrange("(p a) d -> p a d", a=sz),
                    )
                    nrows = [P] * sz
                else:
                    # Final tile(s): ends with the 84-row remainder.
                    full = (ROWS - r0) // P
                    tail = ROWS - r0 - full * P
                    if full:
                        nc.sync.dma_start(
                            out=chunk[:, :full, :],
                            in_=db[r0 : r0 + full * P, :].rearrange(
                                "(a p) d -> p a d", p=P
                            ),
                        )
                    nc.sync.dma_start(
                        out=chunk[:tail, full, :],
                        in_=db[r0 + full * P : ROWS, :],
                    )
                    nrows = [P] * full + [tail]

                for a in range(sz):
                    t = t0 + a
                    rows = nrows[a]
                    sq_v = wpool.tile([P, D_IN], f32, tag="sqv")
                    nc.vector._custom_dve(
                        sqdiff,
                        out=sq_v[:rows, :],
                        in0=chunk[:rows, a, :],
                        in1=qb[:rows, :],
                        accum_out=dis_sb[:rows, t : t + 1],
                    )
                t0 += sz

            # Ship finished distance columns early; only the last few columns
            # remain for the drain. (Valid entries cover every db row; the
            # few never-written slots are ignored by the host.)
            nc.scalar.dma_start(out=dis[:, :64], in_=dis_sb[:, :64])
            nc.scalar.dma_start(out=dis[:, 64:91], in_=dis_sb[:, 64:91])
            nc.scalar.dma_start(out=dis[:, 91:], in_=dis_sb[:, 91:])

    nc.compile()
    _CACHE["nc"] = nc
    return nc


def _run(in_maps, **kwargs):
    from concourse.bass_utils import run_bass_kernel_spmd

    nc = _build()
    return run_bass_kernel_spmd(nc, in_maps, core_ids=list(range(N_CORES)), **kwargs)


def make_in_maps(in_vel, db_vel):
    qb = np.ascontiguousarray(
        np.broadcast_to(np.asarray(in_vel, dtype=np.float32), (P, D_IN))
    )
    return [
        {
            "db": np.ascontiguousarray(db_vel[c * ROWS : (c + 1) * ROWS]),
            "q": qb,
        }
        for c in range(N_CORES)
    ]


def _row_index_map():
    """row_of[p, t] = shard-local db row computed into dis[p, t], or -1."""
    row_of = np.full((P, NTILES), -1, dtype=np.int64)
    p = np.arange(P)
    t0 = 0
    for sz in CHUNKS:
        r0 = t0 * P
        last = t0 + sz == NTILES
        if not last:
            for a in range(sz):
                row_of[:, t0 + a] = r0 + p * sz + a
        else:
            # Final chunk: plain [tail, D] DMA, partition p holds row r0+p.
            tail = ROWS - r0
            row_of[:tail, t0] = r0 + p[:tail]
        t0 += sz
    return row_of


def dense_distances(results):
    """[N_CORES, ROWS] distances in db row order (mirrors the reference)."""
    if "row_of" not in _CACHE:
        _CACHE["row_of"] = _row_index_map()
    row_of = _CACHE["row_of"]
    valid = row_of >= 0
    rows = row_of[valid]
    out = np.empty((N_CORES, ROWS), dtype=np.float32)
    for c in range(N_CORES):
        out[c, rows] = results[c]["dis"][valid]
    return out


def postprocess(results, pred_vel):
    flat = dense_distances(results).reshape(-1)
    ind = int(np.argmin(flat))
    return pred_vel[ind][None, :]


def kernel(in_vel, db_vel, pred_vel):
    res = _run(make_in_maps(in_vel, db_vel))
    return postprocess(res.results, pred_vel)


# revision 14
# speedup vs baseline: 1.2554x; 1.1918x over previous
"""Top-1 nearest-neighbor retrieval kernel for Trainium2 (8 NeuronCores).

Reference computation:
    dis = sum((db_vel - in_vel)**2, axis=1)   # [N]
    ind = argmin(dis)
    out = pred_vel[ind][None, :]

Strategy (memory-bound):
  - Shard db_vel row-wise: 100000 rows -> 8 cores x 12500 rows.
  - Each core streams its 12500x1056 f32 shard from HBM; chunk sizes ramp
    1-2-4 tiles at the start and 4-2-1 at the end (7-tile / 3.78 MB chunks
    in steady state) so compute starts early and drains fast.
  - Distances: each 128-row tile takes ONE fused custom DVE op
        out = (db - q_bcast)**2 ; accum_out = row-sum(out)
    (registered at runtime as SQDIFF_ACC_ANT) - a single 1x-rate vector
    pass (~123 us/core) under the ~130 us DMA stream, so the kernel is
    DMA-bound at ~400 GB/s/core.
  - Each core writes a tiny [128, 98] distance tile; the host does the
    final argmin over 100k scalars and gathers the pred_vel row (pred_vel
    never touches the device - only one of its rows is ever read).
"""

import numpy as np

N_DB = 100000
D_IN = 1056
N_CORES = 8
ROWS = N_DB // N_CORES          # 12500 rows per core
P = 128                         # SBUF partitions
NTILES = (ROWS + P - 1) // P    # 98 tile-columns (97 full + one 84-row)
# Chunk schedule (tiles per DMA): fast ramp-in and drain-out.
CHUNKS = [1, 2, 4] + [7] * 12 + [4, 2, 1]
assert sum(CHUNKS) == NTILES
MAXCHUNK = max(CHUNKS)

_CACHE = {}


def _get_sqdiff_op():
    """Register (once) a fused custom DVE op:
        out = (in0 - in1)**2 ; accum_out = sum(out, axis=free)
    One 1x-rate vector pass computes the whole squared distance."""
    if "op" in _CACHE:
        return _CACHE["op"]

    from operator import add

    from concourse import dve_ops
    from concourse.dve_spec import Spec, Src0, Src1, Zero, _has_src1, lower, sq
    from concourse.dve_uop import DveOpSpec

    NAME = "SQDIFF_ACC_ANT"

    def _ref(in0, in1, c0, c1, c2):
        b = ((in0.astype(np.float32) - in1) ** 2).astype(np.float32)
        return b, b.reshape(b.shape[0], -1).sum(axis=-1, keepdims=True)

    spec = Spec(body=sq(Src0 - Src1), accum=add, accum_init=Zero, reference=_ref)

    existing = {o.name: o for o in dve_ops.OPS}
    if NAME in existing:
        op = existing[NAME]
    else:
        row = max(dve_ops._SUB_OPCODE_FOR_NAME.values()) + 1
        dve_ops._SUB_OPCODE_FOR_NAME[NAME] = row
        shas = {}
        for ver in ("v3", "v4"):
            try:
                uops = lower(spec, ver=ver)
                shas[ver] = DveOpSpec(
                    name=NAME, opcode=row, uops=uops, rd1_en=_has_src1(spec)
                ).sha(ver)
            except Exception:
                pass
        op = dve_ops.DveOp(NAME, spec, subdim=False, uops_sha=shas)
        dve_ops.OPS.append(op)
        dve_ops.CUSTOM_DVE_SPECS[NAME] = spec

    _CACHE["op"] = op
    return op


def _build():
    """Trace + compile the per-core Bass program (cached)."""
    if "nc" in _CACHE:
        return _CACHE["nc"]

    import concourse.bacc as bacc
    import concourse.mybir as mybir
    from concourse.tile import TileContext

    sqdiff = _get_sqdiff_op()
    f32 = mybir.dt.float32

    nc = bacc.Bacc(trn_type="TRN2", debug=False, enable_partition_id=False)
    db = nc.dram_tensor("db", [ROWS, D_IN], f32, kind="ExternalInput").ap()
    q = nc.dram_tensor("q", [P, D_IN], f32, kind="ExternalInput").ap()
    dis = nc.dram_tensor("dis", [P, NTILES], f32, kind="ExternalOutput").ap()

    with TileContext(nc) as tc:
        with (
            tc.tile_pool(name="const", bufs=1) as cpool,
            tc.tile_pool(name="chunks", bufs=4) as chpool,
            tc.tile_pool(name="work", bufs=3) as wpool,
            tc.tile_pool(name="acc", bufs=1) as apool,
        ):
            # Query arrives pre-broadcast to all 128 partitions (host-side).
            qb = cpool.tile([P, D_IN], f32)
            nc.scalar.dma_start(out=qb[:, :], in_=q[:, :])

            dis_sb = apool.tile([P, NTILES], f32)

            t0 = 0
            for sz in CHUNKS:
                r0 = t0 * P
                last = t0 + sz == NTILES
                chunk = chpool.tile([P, MAXCHUNK, D_IN], f32, tag="chunk")
                if not last:
                    # Partition p holds rows r0 + p*sz .. r0 + p*sz + sz-1.
                    nc.sync.dma_start(
                        out=chunk[:, :sz, :],
                        in_=db[r0 : r0 + sz * P, :].rearrange("(p a) d -> p a d", a=sz),
                    )
                    nrows = [P] * sz
                else:
                    # Final tile(s): ends with the 84-row remainder.
                    full = (ROWS - r0) // P
                    tail = ROWS - r0 - full * P
                    if full:
                        nc.sync.dma_start(
                            out=chunk[:, :full, :],
                            in_=db[r0 : r0 + full * P, :].rearrange(
                                "(a p) d -> p a d", p=P
                            ),
                        )
                    nc.sync.dma_start(
                        out=chunk[:tail, full, :],
                        in_=db[r0 + full * P : ROWS, :],
                    )
                    nrows = [P] * full + [tail]

                for a in range(sz):
                    t = t0 + a
                    rows = nrows[a]
                    sq_v = wpool.tile([P, D_IN], f32, tag="sqv")
                    nc.vector._custom_dve(
                        sqdiff,
                        out=sq_v[:rows, :],
                        in0=chunk[:rows, a, :],
                        in1=qb[:rows, :],
                        accum_out=dis_sb[:rows, t : t + 1],
                    )
                t0 += sz

            # Ship finished distance columns early; only the last few columns
            # remain for the drain. (Valid entries cover every db row; the
            # few never-written slots are ignored by the host.)
            nc.scalar.dma_start(out=dis[:, :64], in_=dis_sb[:, :64])
            nc.scalar.dma_start(out=dis[:, 64:91], in_=dis_sb[:, 64:91])
            nc.scalar.dma_start(out=dis[:, 91:], in_=dis_sb[:, 91:])

    nc.compile()
    _CACHE["nc"] = nc
    return nc


def _run(in_maps, **kwargs):
    from concourse.bass_utils import run_bass_kernel_spmd

    nc = _build()
    return run_bass_kernel_spmd(nc, in_maps, core_ids=list(range(N_CORES)), **kwargs)


def make_in_maps(in_vel, db_vel):
    qb = np.ascontiguousarray(
        np.broadcast_to(np.asarray(in_vel, dtype=np.float32), (P, D_IN))
    )
    return [
        {
            "db": np.ascontiguousarray(db_vel[c * ROWS : (c + 1) * ROWS]),
            "q": qb,
        }
        for c in range(N_CORES)
    ]


def _row_index_map():
    """row_of[p, t] = shard-local db row computed into dis[p, t], or -1."""
    row_of = np.full((P, NTILES), -1, dtype=np.int64)
    p = np.arange(P)
    t0 = 0
    for sz in CHUNKS:
        r0 = t0 * P
        last = t0 + sz == NTILES
        if not last:
            for a in range(sz):
                row_of[:, t0 + a] = r0 + p * sz + a
        else:
            # Final chunk: plain [tail, D] DMA, partition p holds row r0+p.
            tail = ROWS - r0
            row_of[:tail, t0] = r0 + p[:tail]
        t0 += sz
    return row_of


def dense_distances(results):
    """[N_CORES, ROWS] distances in db row order (mirrors the reference)."""
    if "row_of" not in _CACHE:
        _CACHE["row_of"] = _row_index_map()
    row_of = _CACHE["row_of"]
    valid = row_of >= 0
    rows = row_of[valid]
    out = np.empty((N_CORES, ROWS), dtype=np.float32)
    for c in range(N_CORES):
        out[c, rows] = results[c]["dis"][valid]
    return out


def postprocess(results, pred_vel):
    flat = dense_distances(results).reshape(-1)
    ind = int(np.argmin(flat))
    return pred_vel[ind][None, :]


def kernel(in_vel, db_vel, pred_vel):
    res = _run(make_in_maps(in_vel, db_vel))
    return postprocess(res.results, pred_vel)


# revision 15
# speedup vs baseline: 1.2585x; 1.0025x over previous
"""Top-1 nearest-neighbor retrieval kernel for Trainium2 (8 NeuronCores).

Reference computation:
    dis = sum((db_vel - in_vel)**2, axis=1)   # [N]
    ind = argmin(dis)
    out = pred_vel[ind][None, :]

Strategy (memory-bound):
  - Shard db_vel row-wise: 100000 rows -> 8 cores x 12500 rows.
  - Each core streams its 12500x1056 f32 shard from HBM; chunk sizes ramp
    1-2-4 tiles at the start and 4-2-1 at the end (7-tile / 3.78 MB chunks
    in steady state) so compute starts early and drains fast.
  - Distances: each 128-row tile takes ONE fused custom DVE op
        out = (db - q_bcast)**2 ; accum_out = row-sum(out)
    (registered at runtime as SQDIFF_ACC_ANT) - a single 1x-rate vector
    pass (~123 us/core) under the ~130 us DMA stream, so the kernel is
    DMA-bound at ~400 GB/s/core.
  - Each core writes a tiny [128, 98] distance tile; the host does the
    final argmin over 100k scalars and gathers the pred_vel row (pred_vel
    never touches the device - only one of its rows is ever read).
"""

import numpy as np

N_DB = 100000
D_IN = 1056
N_CORES = 8
ROWS = N_DB // N_CORES          # 12500 rows per core
P = 128                         # SBUF partitions
NTILES = (ROWS + P - 1) // P    # 98 tile-columns (97 full + one 84-row)
# Chunk schedule (tiles per DMA): fast ramp-in and drain-out.
CHUNKS = [1, 2, 4] + [7] * 12 + [4, 2, 1]
assert sum(CHUNKS) == NTILES
MAXCHUNK = max(CHUNKS)

_CACHE = {}


def _get_sqdiff_op():
    """Register (once) a fused custom DVE op:
        out = (in0 - in1)**2 ; accum_out = sum(out, axis=free)
    One 1x-rate vector pass computes the whole squared distance."""
    if "op" in _CACHE:
        return _CACHE["op"]

    from operator import add

    from concourse import dve_ops
    from concourse.dve_spec import Spec, Src0, Src1, Zero, _has_src1, lower, sq
    from concourse.dve_uop import DveOpSpec

    NAME = "SQDIFF_ACC_ANT"

    def _ref(in0, in1, c0, c1, c2):
        b = ((in0.astype(np.float32) - in1) ** 2).astype(np.float32)
        return b, b.reshape(b.shape[0], -1).sum(axis=-1, keepdims=True)

    spec = Spec(body=sq(Src0 - Src1), accum=add, accum_init=Zero, reference=_ref)

    existing = {o.name: o for o in dve_ops.OPS}
    if NAME in existing:
        op = existing[NAME]
    else:
        row = max(dve_ops._SUB_OPCODE_FOR_NAME.values()) + 1
        dve_ops._SUB_OPCODE_FOR_NAME[NAME] = row
        shas = {}
        for ver in ("v3", "v4"):
            try:
                uops = lower(spec, ver=ver)
                shas[ver] = DveOpSpec(
                    name=NAME, opcode=row, uops=uops, rd1_en=_has_src1(spec)
                ).sha(ver)
            except Exception:
                pass
        op = dve_ops.DveOp(NAME, spec, subdim=False, uops_sha=shas)
        dve_ops.OPS.append(op)
        dve_ops.CUSTOM_DVE_SPECS[NAME] = spec

    _CACHE["op"] = op
    return op


def _build():
    """Trace + compile the per-core Bass program (cached)."""
    if "nc" in _CACHE:
        return _CACHE["nc"]

    import concourse.bacc as bacc
    import concourse.mybir as mybir
    from concourse.tile import TileContext

    sqdiff = _get_sqdiff_op()
    f32 = mybir.dt.float32

    nc = bacc.Bacc(trn_type="TRN2", debug=False, enable_partition_id=False)
    db = nc.dram_tensor("db", [ROWS, D_IN], f32, kind="ExternalInput").ap()
    q = nc.dram_tensor("q", [P, D_IN], f32, kind="ExternalInput").ap()
    dis = nc.dram_tensor("dis", [P, NTILES], f32, kind="ExternalOutput").ap()

    with TileContext(nc) as tc:
        with (
            tc.tile_pool(name="const", bufs=1) as cpool,
            tc.tile_pool(name="chunks", bufs=4) as chpool,
            tc.tile_pool(name="work", bufs=3) as wpool,
            tc.tile_pool(name="acc", bufs=1) as apool,
        ):
            # Query arrives pre-broadcast to all 128 partitions (host-side).
            qb = cpool.tile([P, D_IN], f32)
            nc.scalar.dma_start(out=qb[:, :], in_=q[:, :])

            dis_sb = apool.tile([P, NTILES], f32)

            t0 = 0
            for sz in CHUNKS:
                r0 = t0 * P
                last = t0 + sz == NTILES
                chunk = chpool.tile([P, MAXCHUNK, D_IN], f32, tag="chunk")
                if not last:
                    # Partition p holds rows r0 + p*sz .. r0 + p*sz + sz-1.
                    nc.sync.dma_start(
                        out=chunk[:, :sz, :],
                        in_=db[r0 : r0 + sz * P, :].rearrange("(p a) d -> p a d", a=sz),
                    )
                    nrows = [P] * sz
                else:
                    # Final tile(s): ends with the 84-row remainder.
                    full = (ROWS - r0) // P
                    tail = ROWS - r0 - full * P
                    if full:
                        nc.sync.dma_start(
                            out=chunk[:, :full, :],
                            in_=db[r0 : r0 + full * P, :].rearrange(
                                "(a p) d -> p a d", p=P
                            ),
                        )
                    nc.sync.dma_start(
                        out=chunk[:tail, full, :],
                        in_=db[r0 + full * P : ROWS, :],
                    )
                    nrows = [P] * full + [tail]

                for a in range(sz):
                    t = t0 + a
                    rows = nrows[a]
                    sq_v = wpool.tile([P, D_IN], f32, tag="sqv")
                    nc.vector._custom_dve(
                        sqdiff,
                        out=sq_v[:rows, :],
                        in0=chunk[:rows, a, :],
                        in1=qb[:rows, :],
                        accum_out=dis_sb[:rows, t : t + 1],
                    )
                t0 += sz

            # Ship finished distance columns early; only the last few columns
            # remain for the drain. (Valid entries cover every db row; the
            # few never-written slots are ignored by the host.)
            nc.scalar.dma_start(out=dis[:, :64], in_=dis_sb[:, :64])
            nc.scalar.dma_start(out=dis[:, 64:91], in_=dis_sb[:, 64:91])
            nc.scalar.dma_start(out=dis[:, 91:], in_=dis_sb[:, 91:])

    nc.compile()
    _CACHE["nc"] = nc
    return nc


def _run(in_maps, **kwargs):
    from concourse.bass_utils import run_bass_kernel_spmd

    nc = _build()
    return run_bass_kernel_spmd(nc, in_maps, core_ids=list(range(N_CORES)), **kwargs)


def make_in_maps(in_vel, db_vel):
    qb = np.ascontiguousarray(
        np.broadcast_to(np.asarray(in_vel, dtype=np.float32), (P, D_IN))
    )
    return [
        {
            "db": np.ascontiguousarray(db_vel[c * ROWS : (c + 1) * ROWS]),
            "q": qb,
        }
        for c in range(N_CORES)
    ]


def _row_index_map():
    """row_of[p, t] = shard-local db row computed into dis[p, t], or -1."""
    row_of = np.full((P, NTILES), -1, dtype=np.int64)
    p = np.arange(P)
    t0 = 0
    for sz in CHUNKS:
        r0 = t0 * P
        last = t0 + sz == NTILES
        if not last:
            for a in range(sz):
                row_of[:, t0 + a] = r0 + p * sz + a
        else:
            # Final chunk: plain [tail, D] DMA, partition p holds row r0+p.
            tail = ROWS - r0
            row_of[:tail, t0] = r0 + p[:tail]
        t0 += sz
    return row_of


def dense_distances(results):
    """[N_CORES, ROWS] distances in db row order (mirrors the reference)."""
    if "row_of" not in _CACHE:
        _CACHE["row_of"] = _row_index_map()
    row_of = _CACHE["row_of"]
    valid = row_of >= 0
    rows = row_of[valid]
    out = np.empty((N_CORES, ROWS), dtype=np.float32)
    for c in range(N_CORES):
        out[c, rows] = results[c]["dis"][valid]
    return out


def postprocess(results, pred_vel):
    flat = dense_distances(results).reshape(-1)
    ind = int(np.argmin(flat))
    return pred_vel[ind][None, :]


def kernel(in_vel, db_vel, pred_vel):
    import time

    in_maps = make_in_maps(in_vel, db_vel)
    last_err = None
    for attempt in range(3):
        try:
            res = _run(in_maps)
            return postprocess(res.results, pred_vel)
        except Exception as e:  # transient device/tunnel hiccups
            last_err = e
            time.sleep(2.0 * (attempt + 1))
    raise last_err
